# revision 10
# baseline (speedup 1.0000x reference)
"""M2MRF module as a two-GEMM chained Bass kernel on 8 TRN2 NeuronCores.

Math (per batch b of 4):
    cols = unfold(x[b], k=4, s=4)            # [1024, 16384]
    y1   = W1 @ cols + b1                    # [1024, 16384]
    y2   = W2 @ y1 + b2                      # [256, 16384]
    out[b] = fold(y2, k=2, s=2)              # [64, 256, 256]

Sharding: 8 cores = 4 batches x 2 L-halves (L = 16384 patch positions).
Each core runs GEMM1 (1024x1024x8192) + GEMM2 (256x1024x8192) in bf16
with fp32 PSUM accumulation. Unfold/fold are pure data-movement and run
on the host; the device sees contiguous [K, L] operands resident in SBUF.
"""
import sys

sys.path.insert(0, "/opt/trn_rl_repo")

import numpy as np
import ml_dtypes

import concourse.bass as bass
import concourse.bacc as bacc
import concourse.mybir as mybir
import concourse.tile as tile
from concourse.bass_utils import run_bass_kernel_spmd

P = 128
NT = 512            # free-dim tile (one PSUM bank of fp32)
LSH = 8192          # L per core
NTILES = LSH // NT  # 16
KC = 8              # 1024 / 128 contraction chunks
FC = 1024
COUT = 256

_BF16 = ml_dtypes.bfloat16


def _build_nc(ntiles=NTILES):
    nc = bacc.Bacc("TRN2", target_bir_lowering=False)
    xc_dram = [
        nc.dram_tensor(f"xc{k}", [P, LSH], mybir.dt.bfloat16, kind="ExternalInput")
        for k in range(KC)
    ]
    w1_dram = nc.dram_tensor("w1t", [KC, P, FC], mybir.dt.bfloat16, kind="ExternalInput")
    w2_dram = nc.dram_tensor("w2t", [KC, P, COUT], mybir.dt.bfloat16, kind="ExternalInput")
    y2_dram = nc.dram_tensor("y2", [2, P, LSH], mybir.dt.float32, kind="ExternalOutput")

    with tile.TileContext(nc) as tc:
        with (
            tc.tile_pool(name="resident", bufs=1) as res,
            tc.tile_pool(name="work", bufs=2) as work,
            tc.tile_pool(name="outp", bufs=3) as outp,
            tc.tile_pool(name="ps1", bufs=4, space="PSUM") as ps1,
            tc.tile_pool(name="ps2", bufs=2, space="PSUM") as ps2,
        ):
            w1_sb = res.tile([P, KC, FC], mybir.dt.bfloat16, tag="w1")
            w2_sb = res.tile([P, KC, COUT], mybir.dt.bfloat16, tag="w2")
            for k in range(KC):
                nc.sync.dma_start(w1_sb[:, k, :], w1_dram.ap()[k])
            nc.sync.dma_start(w2_sb[:], w2_dram.ap().rearrange("k p m -> p k m"))

            xc_sb = [
                res.tile([P, LSH], mybir.dt.bfloat16, tag=f"xc{k}", name=f"xc{k}")
                for k in range(KC)
            ]
            # issue loads front-quarter-first across all chunks so the first
            # N-tiles' operands land before the tail of the shard
            for h in range(8):
                sl = slice(h * (LSH // 8), (h + 1) * (LSH // 8))
                for k in range(KC):
                    nc.sync.dma_start(xc_sb[k][:, sl], xc_dram[k].ap()[:, sl])

            for nt in range(ntiles):
                nsl = slice(nt * NT, (nt + 1) * NT)
                y1_sb = work.tile([P, KC, NT], mybir.dt.bfloat16, tag="y1")
                # GEMM1: y1[m,:] = sum_k W1T[k,:,m]^T @ xc[k][:, nsl]
                for m in range(KC):
                    pt = ps1.tile([P, NT], mybir.dt.float32, tag="ps1")
                    for k in range(KC):
                        nc.tensor.matmul(
                            pt[:],
                            w1_sb[:, k, m * P:(m + 1) * P],
                            xc_sb[k][:, nsl],
                            start=(k == 0),
                            stop=(k == KC - 1),
                        )
                    nc.vector.tensor_copy(y1_sb[:, m, :], pt[:])
                # GEMM2: y2[m2,:] = sum_k W2T[k,:,m2]^T @ y1[k,:]
                o_sb = outp.tile([P, 2, NT], mybir.dt.float32, tag="o")
                for m2 in range(2):
                    pt2 = ps2.tile([P, NT], mybir.dt.float32, tag="ps2")
                    for k in range(KC):
                        nc.tensor.matmul(
                            pt2[:],
                            w2_sb[:, k, m2 * P:(m2 + 1) * P],
                            y1_sb[:, k, :],
                            start=(k == 0),
                            stop=(k == KC - 1),
                        )
                    nc.any.tensor_copy(out=o_sb[:, m2, :], in_=pt2[:])
                    nc.sync.dma_start(y2_dram.ap()[m2, :, nsl], o_sb[:, m2, :])

    nc.finalize()
    return nc


_NC_CACHE = None


def kernel(x, W1, b1, W2, b2):
    global _NC_CACHE
    x = np.asarray(x)
    W1, b1 = np.asarray(W1), np.asarray(b1)
    W2, b2 = np.asarray(W2), np.asarray(b2)
    n, c, h, w = x.shape  # 4, 64, 512, 512

    # ---- host unfold: cols[b, c*16+kh*4+kw, ph*128+pw] = x[b,c,ph*4+kh,pw*4+kw]
    xb = x.astype(_BF16)
    cols = xb.reshape(n, c, 128, 4, 128, 4).transpose(0, 1, 3, 5, 2, 4)
    cols = np.ascontiguousarray(cols).reshape(n, 1024, 16384)

    w1t = np.ascontiguousarray(W1.T.astype(_BF16)).reshape(KC, P, FC)
    w2t = np.ascontiguousarray(W2.T.astype(_BF16)).reshape(KC, P, COUT)

    if _NC_CACHE is None:
        _NC_CACHE = _build_nc()
    nc = _NC_CACHE

    in_maps = []
    for core in range(8):
        b, half = core // 2, core % 2
        xc = np.ascontiguousarray(
            cols[b, :, half * LSH:(half + 1) * LSH]
        ).reshape(KC, P, LSH)
        m = {f"xc{k}": xc[k] for k in range(KC)}
        m["w1t"] = w1t
        m["w2t"] = w2t
        in_maps.append(m)

    res = run_bass_kernel_spmd(nc, in_maps, core_ids=list(range(8)))

    # ---- gather + fold on host
    y2 = np.empty((n, COUT, 16384), dtype=np.float32)
    for core in range(8):
        b, half = core // 2, core % 2
        y2[b, :, half * LSH:(half + 1) * LSH] = (
            res.results[core]["y2"].reshape(COUT, LSH)
        )

    # bias epilogue (b1/b2 are zeros in this problem; exact otherwise)
    v = W2.astype(np.float64) @ b1.astype(np.float64) + b2.astype(np.float64)
    if np.any(v):
        y2 += v.astype(np.float32)[None, :, None]

    out = y2.reshape(n, c, 2, 2, 128, 128).transpose(0, 1, 4, 2, 5, 3)
    return np.ascontiguousarray(out).reshape(n, c, 256, 256)


# revision 15
# speedup vs baseline: 1.0221x; 1.0221x over previous
"""M2MRF module as a two-GEMM chained Bass kernel on 8 TRN2 NeuronCores.

Math (per batch b of 4):
    cols = unfold(x[b], k=4, s=4)            # [1024, 16384]
    y1   = W1 @ cols + b1                    # [1024, 16384]
    y2   = W2 @ y1 + b2                      # [256, 16384]
    out[b] = fold(y2, k=2, s=2)              # [64, 256, 256]

Sharding: 8 cores = 4 batches x 2 L-halves (L = 16384 patch positions).
Each core runs GEMM1 (1024x1024x8192) + GEMM2 (256x1024x8192) in bf16
with fp32 PSUM accumulation. Unfold/fold are pure data-movement and run
on the host; the device sees contiguous [K, L] operands resident in SBUF.
"""
import sys

sys.path.insert(0, "/opt/trn_rl_repo")

import numpy as np
import ml_dtypes

import concourse.bass as bass
import concourse.bacc as bacc
import concourse.mybir as mybir
import concourse.tile as tile
from concourse.bass_utils import run_bass_kernel_spmd

P = 128
NT = 512            # free-dim tile (one PSUM bank of fp32)
LSH = 8192          # L per core
NTILES = LSH // NT  # 16
KC = 8              # 1024 / 128 contraction chunks
FC = 1024
COUT = 256

_BF16 = ml_dtypes.bfloat16


def _build_nc(ntiles=NTILES):
    nc = bacc.Bacc("TRN2", target_bir_lowering=False)
    xc_dram = [
        nc.dram_tensor(f"xc{k}", [P, LSH], mybir.dt.bfloat16, kind="ExternalInput")
        for k in range(KC)
    ]
    w1_dram = nc.dram_tensor("w1t", [KC, P, FC], mybir.dt.bfloat16, kind="ExternalInput")  # [m, p, k*128+j]
    w2_dram = nc.dram_tensor("w2t", [KC, P, COUT], mybir.dt.bfloat16, kind="ExternalInput")
    y2_dram = nc.dram_tensor("y2", [2, P, LSH], mybir.dt.float32, kind="ExternalOutput")

    with tile.TileContext(nc) as tc:
        with (
            tc.tile_pool(name="resident", bufs=1) as res,
            tc.tile_pool(name="work", bufs=2) as work,
            tc.tile_pool(name="outp", bufs=3) as outp,
            tc.tile_pool(name="ps1", bufs=4, space="PSUM") as ps1,
            tc.tile_pool(name="ps2", bufs=2, space="PSUM") as ps2,
        ):
            w1_sb = res.tile([P, KC, FC], mybir.dt.bfloat16, tag="w1")
            w2_sb = res.tile([P, KC, COUT], mybir.dt.bfloat16, tag="w2")
            xc_sb = [
                res.tile([P, LSH], mybir.dt.bfloat16, tag=f"xc{k}", name=f"xc{k}")
                for k in range(KC)
            ]
            # Issue order tracks first use: the opening m-group of tile 0 needs
            # only W1's m=0 slice plus the head slice of every x chunk.
            nc.sync.dma_start(w1_sb[:, 0, :], w1_dram.ap()[0])
            hsl = slice(0, LSH // 8)
            for k in range(KC):
                nc.sync.dma_start(xc_sb[k][:, hsl], xc_dram[k].ap()[:, hsl])
            for m in range(1, KC):
                nc.sync.dma_start(w1_sb[:, m, :], w1_dram.ap()[m])
            nc.sync.dma_start(w2_sb[:], w2_dram.ap().rearrange("k p m -> p k m"))
            for h in range(1, 8):
                sl = slice(h * (LSH // 8), (h + 1) * (LSH // 8))
                for k in range(KC):
                    nc.sync.dma_start(xc_sb[k][:, sl], xc_dram[k].ap()[:, sl])

            for nt in range(ntiles):
                nsl = slice(nt * NT, (nt + 1) * NT)
                y1_sb = work.tile([P, KC, NT], mybir.dt.bfloat16, tag="y1")
                # GEMM1: y1[m,:] = sum_k W1T[k,:,m]^T @ xc[k][:, nsl]
                for m in range(KC):
                    pt = ps1.tile([P, NT], mybir.dt.float32, tag="ps1")
                    for k in range(KC):
                        nc.tensor.matmul(
                            pt[:],
                            w1_sb[:, m, k * P:(k + 1) * P],
                            xc_sb[k][:, nsl],
                            start=(k == 0),
                            stop=(k == KC - 1),
                        )
                    nc.vector.tensor_copy(y1_sb[:, m, :], pt[:])
                # GEMM2: y2[m2,:] = sum_k W2T[k,:,m2]^T @ y1[k,:]
                o_sb = outp.tile([P, 2, NT], mybir.dt.float32, tag="o")
                for m2 in range(2):
                    pt2 = ps2.tile([P, NT], mybir.dt.float32, tag="ps2")
                    for k in range(KC):
                        nc.tensor.matmul(
                            pt2[:],
                            w2_sb[:, k, m2 * P:(m2 + 1) * P],
                            y1_sb[:, k, :],
                            start=(k == 0),
                            stop=(k == KC - 1),
                        )
                    nc.any.tensor_copy(out=o_sb[:, m2, :], in_=pt2[:])
                    nc.sync.dma_start(y2_dram.ap()[m2, :, nsl], o_sb[:, m2, :])

    nc.finalize()
    return nc


_NC_CACHE = None


def kernel(x, W1, b1, W2, b2):
    global _NC_CACHE
    x = np.asarray(x)
    W1, b1 = np.asarray(W1), np.asarray(b1)
    W2, b2 = np.asarray(W2), np.asarray(b2)
    n, c, h, w = x.shape  # 4, 64, 512, 512

    # ---- host unfold: cols[b, c*16+kh*4+kw, ph*128+pw] = x[b,c,ph*4+kh,pw*4+kw]
    xb = x.astype(_BF16)
    cols = xb.reshape(n, c, 128, 4, 128, 4).transpose(0, 1, 3, 5, 2, 4)
    cols = np.ascontiguousarray(cols).reshape(n, 1024, 16384)

    w1t = np.ascontiguousarray(
        W1.astype(_BF16).reshape(KC, P, KC, P).transpose(0, 3, 2, 1)
    ).reshape(KC, P, FC)
    w2t = np.ascontiguousarray(W2.T.astype(_BF16)).reshape(KC, P, COUT)

    if _NC_CACHE is None:
        _NC_CACHE = _build_nc()
    nc = _NC_CACHE

    in_maps = []
    for core in range(8):
        b, half = core // 2, core % 2
        xc = np.ascontiguousarray(
            cols[b, :, half * LSH:(half + 1) * LSH]
        ).reshape(KC, P, LSH)
        m = {f"xc{k}": xc[k] for k in range(KC)}
        m["w1t"] = w1t
        m["w2t"] = w2t
        in_maps.append(m)

    res = run_bass_kernel_spmd(nc, in_maps, core_ids=list(range(8)))

    # ---- gather + fold on host
    y2 = np.empty((n, COUT, 16384), dtype=np.float32)
    for core in range(8):
        b, half = core // 2, core % 2
        y2[b, :, half * LSH:(half + 1) * LSH] = (
            res.results[core]["y2"].reshape(COUT, LSH)
        )

    # bias epilogue (b1/b2 are zeros in this problem; exact otherwise)
    v = W2.astype(np.float64) @ b1.astype(np.float64) + b2.astype(np.float64)
    if np.any(v):
        y2 += v.astype(np.float32)[None, :, None]

    out = y2.reshape(n, c, 2, 2, 128, 128).transpose(0, 1, 4, 2, 5, 3)
    return np.ascontiguousarray(out).reshape(n, c, 256, 256)
